# revision 1
# baseline (speedup 1.0000x reference)
"""Bass/Trainium2 kernel for nn_DenoisedSasrec (sparse attention SASRec block).

Data-parallel over batch: 32 sequences -> 8 NeuronCores x 4 sequences.

Math (per sequence, reference semantics):
    x  = item_emb[positives] + pos_emb                     (L, D)
    z  = silu(x @ Wz^T);  v = silu(x @ Wv^T)               (L, E) / (L, D)
    s  = (z @ Wq^T * gq) @ (z @ Wk^T * gk)^T  == z @ A @ z^T,
         A = Wq^T diag(gq*gk) Wk               (betas are zero)
    a  = relu(s * m * sparse_w)^2 / (L*D),
         m[i,j] = mask[j] for j<i, 1 for j==i, 0 for j>i
    out = a @ v                                            (L, D)

Folding: W2T[j,i] = (j<=i) * sparse_w[i,j] / sqrt(L*D) (host),
         dw1m[b,j] = sparse_w[j,j]*(1-mask[b,j]) / sqrt(L*D) (host),
so a^T[j,i] = relu(s^T[j,i]*W2T[j,i]*mask[b,j] + eye*s^T*dw1m)^2.

On-chip layouts (partition dim first):
    xT[d',l], zT[e,l], tT[e,i] (t = z@A), v[j,d] natural, sT/aT[j,i].
All matmuls issued as float32r (full PE rate at N>=512, ~fp32 accuracy).
"""

import os
import sys

import numpy as np

for _p in ("/opt/trn_rl_repo", "/root/.axon_site/_ro/trn_rl_repo"):
    if os.path.isdir(_p) and _p not in sys.path:
        sys.path.append(_p)

B, L, D = 32, 1024, 1024
N_ITEMS = 50000
NCORES = 8
BPC = B // NCORES          # sequences per core
P = 128
NT = L // P                # 8 partition tiles per 1024 dim
NS = L // 512              # 2 free-dim slices of 512

_CACHE = {}


def _build_bass():
    import concourse.bass as bass
    import concourse.bacc as bacc
    import concourse.mybir as mybir
    import concourse.tile as tile
    from concourse.bass import ts, ds
    from concourse.masks import make_identity
    from contextlib import ExitStack

    f32 = mybir.dt.float32
    f32r = mybir.dt.float32r
    i32 = mybir.dt.int32
    AF = mybir.ActivationFunctionType
    OP = mybir.AluOpType

    nc = bacc.Bacc("TRN2", target_bir_lowering=False, debug=False, num_devices=NCORES)

    emb_h = nc.declare_dram_parameter("emb", [N_ITEMS + 1, D], f32, isOutput=False)
    pos_h = nc.declare_dram_parameter("posm", [L, D], f32, isOutput=False)
    idx_h = nc.declare_dram_parameter("idx", [P, BPC * NT], i32, isOutput=False)
    wz_h = nc.declare_dram_parameter("wzT", [D, L], f32r, isOutput=False)
    wv_h = nc.declare_dram_parameter("wvT", [D, D], f32r, isOutput=False)
    am_h = nc.declare_dram_parameter("amat", [L, L], f32r, isOutput=False)
    w2_h = nc.declare_dram_parameter("w2T", [L, L], f32, isOutput=False)
    mk_h = nc.declare_dram_parameter("maskv", [BPC, P, NT], f32, isOutput=False)
    dw_h = nc.declare_dram_parameter("dw1m", [BPC, P, NT], f32, isOutput=False)
    out_h = nc.declare_dram_parameter("out", [BPC * L, D], f32, isOutput=True)

    def mm(ps, lhsT, rhs, start, stop):
        nc.tensor.matmul(out=ps, lhsT=lhsT, rhs=rhs, start=start, stop=stop)

    with ExitStack() as ctx:
        tc = ctx.enter_context(tile.TileContext(nc))

        const_p = ctx.enter_context(tc.tile_pool(name="const", bufs=1))
        small_p = ctx.enter_context(tc.tile_pool(name="small", bufs=4))
        xg_p = ctx.enter_context(tc.tile_pool(name="xg", bufs=3))
        xt_p = ctx.enter_context(tc.tile_pool(name="xtt", bufs=NT))
        zt_p = ctx.enter_context(tc.tile_pool(name="zt", bufs=NT))
        tt_p = xt_p
        v_p = ctx.enter_context(tc.tile_pool(name="v", bufs=NT))
        w_p = ctx.enter_context(tc.tile_pool(name="w", bufs=12))
        at_p = ctx.enter_context(tc.tile_pool(name="at", bufs=NT))
        sc_p = ctx.enter_context(tc.tile_pool(name="sc", bufs=3))
        ob_p = ctx.enter_context(tc.tile_pool(name="ob", bufs=2))
        ps_all = ctx.enter_context(tc.tile_pool(name="ps_all", bufs=8, space="PSUM"))
        ps_tr = ps_all
        ps_mm = ps_all
        ps_s = ps_all
        ps_o = ps_all

        ident = const_p.tile([P, P], f32)
        make_identity(nc, ident[:])

        # all gather indices, loaded once on the SWDGE queue (first: the
        # batch-0 gathers are the kernel's critical-path start)
        it_all = const_p.tile([P, BPC * NT], i32)
        nc.gpsimd.dma_start(out=it_all[:], in_=idx_h[:])
        w2touch = const_p.tile([P, NT], f32)

        def emit_gathers(b):
            # gather + pos-add for batch b; emitted one phase early so the
            # SWDGE queue and the DVE adds overlap the previous batch's
            # projection matmuls
            tiles = []
            for l_t in range(NT):
                xg = xg_p.tile([P, D], f32, tag="xg", bufs=8,
                               name=f"xg_{b}_{l_t}")
                # fill with pos_emb rows on the HWDGE ring, then gather-add
                # the embedding rows on top (CCE add inside the gather DMA)
                nc.scalar.dma_start(out=xg[:], in_=pos_h[ts(l_t, P), :])
                nc.gpsimd.indirect_dma_start(
                    out=xg[:],
                    out_offset=None,
                    in_=emb_h[:],
                    in_offset=bass.IndirectOffsetOnAxis(
                        ap=it_all[:, b * NT + l_t : b * NT + l_t + 1], axis=0
                    ),
                    compute_op=OP.add,
                )
                tiles.append(xg)
            return tiles

        xg_next = emit_gathers(0)

        # PE warm-up: keep the PE busy (HAM un-throttle) while batch-0
        # gathers stream in; results are discarded
        warm_ps = ps_tr.tile([P, 512], f32, space="PSUM", tag="ps", name="warm_ps")
        for wi in range(24):
            nc.tensor.transpose(
                out=warm_ps[:, ts(wi % 4, P)], in_=ident[:], identity=ident[:]
            )

        for b in range(BPC):
            maskv = small_p.tile([P, NT], f32, tag="maskv")
            nc.sync.dma_start(out=maskv[:], in_=mk_h[b])
            dwv = small_p.tile([P, NT], f32, tag="dwv")
            nc.sync.dma_start(out=dwv[:], in_=dw_h[b])

            # ---- gather + transpose: xT[d_t][:, l] = emb[idx[l], d] + pos[l, d]
            xT = [xt_p.tile([P, L], f32r, tag="xt", name=f"xT_{b}_{j}") for j in range(NT)]
            xg_cur, xg_next = xg_next, None
            for l_t in range(NT):
                xg = xg_cur[l_t]
                # transpose 8 [128,128] blocks -> two [128,512] psum tiles,
                # evacuated on ACT in [128,128] slices into the xT tiles
                # (bacc's event-semaphore pass legalizes the multi-wait
                # transposes for walrus codegen)
                for ps_i in range(2):
                    pt = ps_tr.tile([P, 512], f32, space="PSUM", tag="ps")
                    for k in range(4):
                        d_t = ps_i * 4 + k
                        nc.tensor.transpose(
                            out=pt[:, ts(k, P)], in_=xg[:, ts(d_t, P)],
                            identity=ident[:],
                        )
                    for k in range(4):
                        d_t = ps_i * 4 + k
                        nc.scalar.copy(
                            out=xT[d_t][:, ts(l_t, P)], in_=pt[:, ts(k, P)]
                        )

            # ---- z = silu(x @ Wz^T) as zT[e, l]
            wz_t = [w_p.tile([P, L], f32r, tag="w", name=f"wz_{b}_{j}") for j in range(NT)]
            for d_t in range(NT):
                nc.scalar.dma_start(out=wz_t[d_t][:], in_=wz_h[ts(d_t, P), :])
            zT = [zt_p.tile([P, L], f32r, tag="zt", name=f"zT_{b}_{j}") for j in range(NT)]
            for l_s in range(NS):
                for e_t in range(NT):
                    ps = ps_mm.tile([P, 512], f32, space="PSUM", tag="ps")
                    for d_t in range(NT):
                        mm(ps[:], wz_t[d_t][:, ts(e_t, P)], xT[d_t][:, ts(l_s, 512)],
                           d_t == 0, d_t == NT - 1)
                    nc.scalar.activation(
                        out=zT[e_t][:, ts(l_s, 512)], in_=ps[:], func=AF.Silu
                    )

            # ---- v = silu(x @ Wv^T), natural layout v[j, d]
            wv_t = [w_p.tile([P, L], f32r, tag="w", name=f"wv_{b}_{j}") for j in range(NT)]
            for d_t in range(NT):
                nc.scalar.dma_start(out=wv_t[d_t][:], in_=wv_h[ts(d_t, P), :])
            v_t = [v_p.tile([P, D], f32r, tag="v", name=f"v_{b}_{j}") for j in range(NT)]
            for j_t in range(NT):
                for d_s in range(NS):
                    ps = ps_mm.tile([P, 512], f32, space="PSUM", tag="ps")
                    for d_t in range(NT):
                        mm(ps[:], xT[d_t][:, ts(j_t, P)], wv_t[d_t][:, ts(d_s, 512)],
                           d_t == 0, d_t == NT - 1)
                    nc.scalar.activation(
                        out=v_t[j_t][:, ts(d_s, 512)], in_=ps[:], func=AF.Silu
                    )

            # ---- t = z @ A as tT[e, i]
            am_t = [w_p.tile([P, L], f32r, tag="w", name=f"am_{b}_{j}") for j in range(NT)]
            for e2 in range(NT):
                nc.scalar.dma_start(out=am_t[e2][:], in_=am_h[ts(e2, P), :])
            tT = [tt_p.tile([P, L], f32r, tag="xt", name=f"tT_{b}_{j}") for j in range(NT)]
            for i_s in range(NS):
                for e_t in range(NT):
                    ps = ps_mm.tile([P, 512], f32, space="PSUM", tag="ps")
                    for e2 in range(NT):
                        mm(ps[:], am_t[e2][:, ts(e_t, P)], zT[e2][:, ts(i_s, 512)],
                           e2 == 0, e2 == NT - 1)
                    nc.vector.tensor_copy(
                        out=tT[e_t][:, ts(i_s, 512)], in_=ps[:]
                    )

            # stream W2T for this batch's attention phase through the
            # weight pool (full double-buffer rotation wz->wv->am->w2)
            w2_t = [w_p.tile([P, L], f32, tag="w", name=f"w2_{b}_{j}") for j in range(NT)]
            for j in range(NT):
                nc.scalar.dma_start(out=w2_t[j][:], in_=w2_h[ts(j, P), :])
            for j in range(NT):
                nc.vector.tensor_copy(
                    out=w2touch[:, j : j + 1], in_=w2_t[j][:, 0:1]
                )
            if b + 1 < BPC:
                xg_next = emit_gathers(b + 1)

            # ---- attention: sT[j,i] = sum_e zT[e,j] tT[e,i]; mask; relu^2;
            # out rows are emitted as soon as their last aT tile is masked,
            # so PE alternates s-matmuls with out-matmuls instead of
            # waiting on the DVE mask pipeline.
            for i_s in range(NS):
                n_jt = 4 if i_s == 0 else NT  # causal: skip j-tiles above diag
                aT = [at_p.tile([P, 512], f32r, tag="at", name=f"aT_{b}_{i_s}_{j}") for j in range(n_jt)]
                for j_t in range(n_jt):
                    # exact causal column start for this j-tile, floored so
                    # the moving dim stays >=256 (full f32r rate); columns
                    # below `off` are strictly-future (a==0) and are never
                    # read by the causally-skipped out-matmuls
                    off = min(max(j_t * P - i_s * 512, 0), 256)
                    w_c = 512 - off
                    ao = i_s * 512 + off
                    sps = ps_s.tile([P, 512], f32, space="PSUM", tag="ps")
                    for e_t in range(NT):
                        mm(sps[:, :w_c], zT[e_t][:, ts(j_t, P)],
                           tT[e_t][:, ds(ao, w_c)],
                           e_t == 0, e_t == NT - 1)
                    u = sc_p.tile([P, 512], f32, tag="sc", bufs=3)
                    nc.vector.tensor_tensor(
                        out=u[:, :w_c], in0=sps[:, :w_c],
                        in1=w2_t[j_t][:, ds(ao, w_c)],
                        op=OP.mult,
                    )
                    # diagonal 128-block of this j_t, if inside this i-slice
                    dsub = j_t * P - i_s * 512 - off
                    if 0 <= dsub < w_c:
                        dg = sc_p.tile([P, P], f32, tag="dg", bufs=3)
                        nc.vector.tensor_tensor(
                            out=dg[:], in0=sps[:, ds(dsub, P)], in1=ident[:],
                            op=OP.mult,
                        )
                        nc.vector.tensor_scalar_mul(
                            out=dg[:], in0=dg[:], scalar1=dwv[:, j_t : j_t + 1]
                        )
                        nc.vector.tensor_scalar_mul(
                            out=u[:, :w_c], in0=u[:, :w_c],
                            scalar1=maskv[:, j_t : j_t + 1]
                        )
                        nc.vector.tensor_add(
                            out=u[:, ds(dsub, P)], in0=u[:, ds(dsub, P)], in1=dg[:]
                        )
                        nc.scalar.activation(
                            out=u[:, :w_c], in_=u[:, :w_c], func=AF.Relu
                        )
                    else:
                        nc.vector.tensor_scalar(
                            out=u[:, :w_c], in0=u[:, :w_c],
                            scalar1=maskv[:, j_t : j_t + 1], scalar2=0.0,
                            op0=OP.mult, op1=OP.max,
                        )
                    nc.vector.tensor_tensor(
                        out=aT[j_t][:, ds(off, w_c)], in0=u[:, :w_c],
                        in1=u[:, :w_c], op=OP.mult
                    )

                    # emit the out row lagged by one j-tile, so the DVE
                    # mask chain for aT[j_t] hides behind the next s-matmul
                    emit_igs = [j_t - 1] if j_t - 1 >= i_s * 4 else []
                    if j_t == n_jt - 1:
                        emit_igs.append(j_t)
                    for ig in emit_igs:
                        i_t = ig - i_s * 4
                        for d_s in range(NS):
                            ops = ps_o.tile([P, 512], f32, space="PSUM", tag="ps")
                            for j2 in range(ig + 1):
                                mm(ops[:], aT[j2][:, ts(i_t, P)],
                                   v_t[j2][:, ts(d_s, 512)],
                                   j2 == 0, j2 == ig)
                            ot = ob_p.tile([P, 512], f32, tag="ob", bufs=3)
                            nc.scalar.copy(out=ot[:], in_=ops[:])
                            nc.sync.dma_start(
                                out=out_h[b * L + ig * P : b * L + (ig + 1) * P,
                                          ts(d_s, 512)],
                                in_=ot[:],
                            )

    nc.compile()
    return nc


def _host_prep(positives, mask, item_emb, pos_emb, Wz, Wv, Wq, Wk,
               gamma_q, beta_q, gamma_k, beta_k, sparse_w):
    gq, gk = np.asarray(gamma_q[0]), np.asarray(gamma_k[0])
    bq, bk = np.asarray(beta_q[0]), np.asarray(beta_k[0])
    assert np.abs(bq).max() == 0.0 and np.abs(bk).max() == 0.0, (
        "kernel assumes beta_q/beta_k == 0 (true for this model's init)"
    )
    Wq = np.asarray(Wq, np.float32)
    Wk = np.asarray(Wk, np.float32)
    sw = np.asarray(sparse_w, np.float32)
    scale = 1.0 / np.float32(np.sqrt(float(L) * float(D)))

    amat = ((Wq.T * (gq * gk)[None, :].astype(np.float32)) @ Wk).astype(np.float32)
    w2T = (np.triu(np.ones((L, L), np.float32)) * sw.T * scale).astype(np.float32)
    mk = np.asarray(mask, np.float32)                       # (B, L)
    dw = (np.diag(sw) * scale)[None, :] * (1.0 - mk)        # (B, L)

    # idx[c][p, b*NT+t] = positives[4c+b, t*128+p]
    idx = np.ascontiguousarray(
        np.asarray(positives).astype(np.int32)
        .reshape(NCORES, BPC, NT, P).transpose(0, 3, 1, 2)
        .reshape(NCORES, P, BPC * NT)
    )
    maskv = np.ascontiguousarray(mk.reshape(B, NT, P).transpose(0, 2, 1))
    dwv = np.ascontiguousarray(dw.astype(np.float32).reshape(B, NT, P).transpose(0, 2, 1))

    common = {
        "emb": np.ascontiguousarray(np.asarray(item_emb, np.float32)),
        "posm": np.ascontiguousarray(np.asarray(pos_emb, np.float32)),
        "wzT": np.ascontiguousarray(np.asarray(Wz, np.float32).T),
        "wvT": np.ascontiguousarray(np.asarray(Wv, np.float32).T),
        "amat": np.ascontiguousarray(amat),
        "w2T": np.ascontiguousarray(w2T),
    }
    in_maps = []
    for c in range(NCORES):
        sl = slice(c * BPC, (c + 1) * BPC)
        m = dict(common)
        m["idx"] = np.ascontiguousarray(idx[c])
        m["maskv"] = np.ascontiguousarray(maskv[sl])
        m["dw1m"] = np.ascontiguousarray(dwv[sl])
        in_maps.append(m)
    return in_maps


def _get_nc():
    if "nc" not in _CACHE:
        _CACHE["nc"] = _build_bass()
    return _CACHE["nc"]


def kernel(**inputs) -> np.ndarray:
    from concourse.bass_utils import run_bass_kernel_spmd

    in_maps = _host_prep(**inputs)
    nc = _get_nc()
    res = run_bass_kernel_spmd(
        nc, in_maps, core_ids=list(range(NCORES)),
        **_CACHE.get("run_kwargs", {}),
    )
    out = np.concatenate(
        [r["out"].reshape(BPC, L, D) for r in res.results], axis=0
    )
    _CACHE["last_results"] = res
    return out


if __name__ == "__main__":
    # smoke: build only
    nc = _get_nc()
    print("built bass module OK")

